# revision 5
# baseline (speedup 1.0000x reference)
"""DiffHead kernel for 8 Trainium2 NeuronCores.

Data-parallel over the batch dim (32 images -> 4 per core). Per image, the
whole pipeline runs on-chip:
  depthwise conv1d + mixer + encoder (fp32 DVE/ACT + fp32 matmuls, N=288)
  pairwise cosine similarity via Gram matmuls (fp32, exact)
  stem conv2d (6->256) via K=54 im2col matmul (fp32r)
  residual block: two 256->256 3x3 convs as 9-tap shifted fp32r matmuls
      accumulated in PSUM, using a flat row-padded layout (stride 98, base 2)
      so every tap is a constant 1D shift with even PSUM offsets
  SE + fuse head batched over the core's 4 images (N=4)

Weights for the two big convs are streamed from DRAM per (conv, m-block).
"""
import sys
import os

sys.path.insert(0, "/opt/trn_rl_repo")
os.environ.setdefault("JAX_PLATFORMS", "cpu")

import numpy as np
from contextlib import ExitStack

import concourse.bass as bass
from concourse import bacc
import concourse.tile as tile
from concourse import mybir

F32 = mybir.dt.float32
F32R = mybir.dt.float32r
AF = mybir.ActivationFunctionType
ALU = mybir.AluOpType
AX = mybir.AxisListType

DIM = 256
NL = 3
T = 96
BNW = 32
EPS_BN = 1e-5
EPS_COS = 1e-8

NCORES = 8
IPC = BNW // NCORES          # images per core = 4
S = T + 2                    # flat row stride = 98
BASE = 2                     # head pad so shifted reads stay in-bounds
FLW = BASE + T * S + 2       # flat layout width = 9412
COLT = 4 * S                 # 392 columns per conv matmul tile (4 rows)
NCT = T * S // COLT          # 24 column tiles
L3 = NL * T                  # 288

_COLS = {}


def _col(name):
    if name not in _COLS:
        _COLS[name] = len(_COLS)
    return _COLS[name]


def _sp_layout():
    for i in range(3):
        for j in range(3):
            _col(f"tw{i}{j}")
        _col(f"tb{i}")
    for n in ("s_pre", "b_pre", "s_mx0", "b_mx0", "s_mx1", "b_mx1"):
        _col(n)
    for j in range(2):
        for t_ in range(3):
            _col(f"edw{j}{t_}")
        _col(f"edb{j}")
        _col(f"s_enc{j}")
        _col(f"b_enc{j}")
    for n in ("s_st", "b_st", "s_b1", "b_b1", "s_b2", "b_b2", "fuse_b"):
        _col(n)
    return len(_COLS)


NSP = _sp_layout()

# conv taps ordered center-first (the center tap covers the full PSUM range,
# so it carries start=True and every later partial-range tap accumulates)
TAPS = [(1, 1)] + [(dh, dw) for dh in range(3) for dw in range(3)
                   if (dh, dw) != (1, 1)]


def build_program():
    nc = bacc.Bacc("TRN2", target_bir_lowering=False, debug=False)

    x_d = nc.declare_dram_parameter("x", [NL, IPC, DIM, T], F32, isOutput=False)
    convw_d = nc.declare_dram_parameter("convw", [2, 2, 128, 2, 9, 128], F32R,
                                        isOutput=False)
    stemw_d = nc.declare_dram_parameter("stemw", [54, 256], F32R, isOutput=False)
    mxw0_d = nc.declare_dram_parameter("mxw0", [128, 2, 256], F32, isOutput=False)
    mxw1_d = nc.declare_dram_parameter("mxw1", [128, 2, 256], F32, isOutput=False)
    encw0_d = nc.declare_dram_parameter("encw0", [128, 2, 256], F32, isOutput=False)
    encw1_d = nc.declare_dram_parameter("encw1", [128, 2, 256], F32, isOutput=False)
    fusew_d = nc.declare_dram_parameter("fusew", [128, 4, 256], F32, isOutput=False)
    se1w1_d = nc.declare_dram_parameter("se1w1", [128, 2, 64], F32, isOutput=False)
    se1w2_d = nc.declare_dram_parameter("se1w2", [64, 256], F32, isOutput=False)
    se2w1_d = nc.declare_dram_parameter("se2w1", [128, 2, 64], F32, isOutput=False)
    se2w2_d = nc.declare_dram_parameter("se2w2", [64, 256], F32, isOutput=False)
    sp_d = nc.declare_dram_parameter("sp", [128, 2, NSP], F32, isOutput=False)
    out_d = nc.declare_dram_parameter("out", [IPC, DIM], F32, isOutput=True)

    with tile.TileContext(nc) as tc, ExitStack() as ctx:
        fixed = ctx.enter_context(tc.tile_pool(name="fixed", bufs=1))
        scr = ctx.enter_context(tc.tile_pool(name="scr", bufs=2))
        wst = ctx.enter_context(tc.tile_pool(name="wst", bufs=2))
        img = ctx.enter_context(tc.tile_pool(name="img", bufs=1))
        ep = ctx.enter_context(tc.tile_pool(name="ep", bufs=2))
        psC = ctx.enter_context(tc.tile_pool(name="psC", bufs=4, space="PSUM"))
        psG = ctx.enter_context(tc.tile_pool(name="psG", bufs=3, space="PSUM"))

        # ---- persistent loads ----
        stemw_t = fixed.tile([54, 256], F32R)
        nc.sync.dma_start(stemw_t[:], stemw_d[:])
        mxw = []
        for d in (mxw0_d, mxw1_d, encw0_d, encw1_d):
            t_ = fixed.tile([128, 2, 256], F32, tag=f"w_{d.name}")
            nc.sync.dma_start(t_[:], d[:])
            mxw.append(t_)
        mxw0_t, mxw1_t, encw0_t, encw1_t = mxw
        fusew_t = fixed.tile([128, 4, 256], F32)
        nc.sync.dma_start(fusew_t[:], fusew_d[:])
        se1w1_t = fixed.tile([128, 2, 64], F32)
        nc.sync.dma_start(se1w1_t[:], se1w1_d[:])
        se1w2_t = fixed.tile([64, 256], F32)
        nc.sync.dma_start(se1w2_t[:], se1w2_d[:])
        se2w1_t = fixed.tile([128, 2, 64], F32)
        nc.sync.dma_start(se2w1_t[:], se2w1_d[:])
        se2w2_t = fixed.tile([64, 256], F32)
        nc.sync.dma_start(se2w2_t[:], se2w2_d[:])
        sp_t = fixed.tile([128, 2, NSP], F32)
        nc.sync.dma_start(sp_t[:], sp_d[:])

        def spc(name, blk):
            return sp_t[:, blk, _col(name):_col(name) + 1]

        ones_t = fixed.tile([128, 1], F32)
        nc.vector.memset(ones_t[:], 1.0)
        zero_t = fixed.tile([128, S], F32)
        nc.vector.memset(zero_t[:], 0.0)

        def zero_pads(dst2d, nparts):
            """Zero the non-interior cells of one flat-layout plane."""
            nc.vector.tensor_copy(dst2d[0:nparts, 0:BASE],
                                  zero_t[0:nparts, 0:BASE])
            nc.vector.tensor_copy(dst2d[0:nparts, FLW - 2:FLW],
                                  zero_t[0:nparts, 0:2])
            rows = dst2d[0:nparts, BASE:BASE + T * S].rearrange(
                "p (h s) -> p h s", h=T)
            zcol = zero_t[0:nparts, 0:T].rearrange("p (h o) -> p h o", h=T)
            nc.vector.tensor_copy(rows[:, :, 0:1], zcol)
            nc.vector.tensor_copy(rows[:, :, S - 1:S], zcol)

        # h: stem output, fp32r flat-padded; pads zeroed once (interior is
        # rewritten every image, pads never touched again)
        h_t = fixed.tile([128, 2, FLW], F32R)
        for blk in range(2):
            zero_pads(h_t[:, blk, :], 128)

        pooled_d1d = fixed.tile([128, 2, IPC], F32)
        pooled_dm = fixed.tile([128, 2, IPC], F32)

        for b in range(IPC):
            # ---- load x (3 levels, 2 channel blocks) ----
            xin = img.tile([128, 2, L3], F32, tag="xin")
            for l in range(NL):
                for blk in range(2):
                    nc.sync.dma_start(
                        xin[:, blk, l * T:(l + 1) * T],
                        x_d[l, b, blk * 128:(blk + 1) * 128, :])

            # ---- stage A: depthwise conv1d k=3 + relu, feats = 0.5*(t+x) ----
            feats = img.tile([128, 2, L3], F32, tag="feats")
            tmp96 = img.tile([128, 2, T], F32, tag="tmp96")

            def dwconv(dst, src, wname, bname, blk, l):
                o = l * T
                xs = src[:, blk, o:o + T]
                ts_ = dst[:, blk, o:o + T]
                nc.vector.tensor_scalar(ts_, xs, spc(wname + "1", blk),
                                        spc(bname, blk), ALU.mult, ALU.add)
                tm = tmp96[:, blk, :]
                nc.vector.tensor_scalar(tm[0:128, 0:T - 1],
                                        src[:, blk, o + 1:o + T],
                                        spc(wname + "2", blk), None, ALU.mult)
                nc.vector.tensor_tensor(ts_[0:128, 0:T - 1], ts_[0:128, 0:T - 1],
                                        tm[0:128, 0:T - 1], ALU.add)
                nc.vector.tensor_scalar(tm[0:128, 0:T - 1],
                                        src[:, blk, o:o + T - 1],
                                        spc(wname + "0", blk), None, ALU.mult)
                nc.vector.tensor_tensor(ts_[0:128, 1:T], ts_[0:128, 1:T],
                                        tm[0:128, 0:T - 1], ALU.add)

            for blk in range(2):
                for l in range(NL):
                    dwconv(feats, xin, f"tw{l}", f"tb{l}", blk, l)
                    ts_ = feats[:, blk, l * T:(l + 1) * T]
                    xs = xin[:, blk, l * T:(l + 1) * T]
                    nc.vector.tensor_scalar(ts_, ts_, 0.0, None, ALU.max)
                    nc.vector.tensor_tensor(ts_, ts_, xs, ALU.add)
                    nc.vector.tensor_scalar(ts_, ts_, 0.5, None, ALU.mult)

            # ---- stage B: DiffMixer ----
            xn = img.tile([128, 2, L3], F32, tag="enc")
            dp = img.tile([128, 2, L3], F32, tag="dwt")
            for blk in range(2):
                nc.vector.tensor_scalar(xn[:, blk, :], feats[:, blk, :],
                                        spc("s_pre", blk), spc("b_pre", blk),
                                        ALU.mult, ALU.add)
            for blk in range(2):
                for l in range(NL):
                    o = l * T
                    nc.vector.tensor_tensor(dp[:, blk, o + 1:o + T],
                                            xn[:, blk, o + 1:o + T],
                                            xn[:, blk, o:o + T - 1],
                                            ALU.subtract)
                    nc.vector.tensor_tensor(dp[:, blk, o:o + 1],
                                            xn[:, blk, o + 1:o + 2],
                                            xn[:, blk, o:o + 1], ALU.subtract)

            xmix = img.tile([128, 2, L3], F32, tag="xmix")
            dtmp = img.tile([128, 2, L3], F32, tag="dtmp")
            for src, wt_, sn, bn_ in ((xn, mxw0_t, "s_mx0", "b_mx0"),
                                      (dp, mxw1_t, "s_mx1", "b_mx1")):
                for m in range(2):
                    pm = psG.tile([128, L3], F32, tag="psg")
                    for k in range(2):
                        nc.tensor.matmul(pm[:], wt_[:, k, m * 128:(m + 1) * 128],
                                         src[:, k, :], start=(k == 0),
                                         stop=(k == 1))
                    nc.scalar.activation(dtmp[:, m, :], pm[:], AF.Gelu,
                                         bias=spc(bn_, m), scale=spc(sn, m))
                base = feats if src is xn else xmix
                for m in range(2):
                    nc.vector.tensor_tensor(xmix[:, m, :], base[:, m, :],
                                            dtmp[:, m, :], ALU.add)

            # ---- stage C: encoder ----
            cur = xmix
            for j in range(2):
                dwt = img.tile([128, 2, L3], F32, tag="dwt")
                for blk in range(2):
                    for l in range(NL):
                        dwconv(dwt, cur, f"edw{j}", f"edb{j}", blk, l)
                enc = img.tile([128, 2, L3], F32, tag="enc")
                wt_ = encw0_t if j == 0 else encw1_t
                for m in range(2):
                    pm = psG.tile([128, L3], F32, tag="psg")
                    for k in range(2):
                        nc.tensor.matmul(pm[:], wt_[:, k, m * 128:(m + 1) * 128],
                                         dwt[:, k, :], start=(k == 0),
                                         stop=(k == 1))
                    nc.scalar.activation(enc[:, m, :], pm[:], AF.Relu,
                                         bias=spc(f"b_enc{j}", m),
                                         scale=spc(f"s_enc{j}", m))
                cur = enc

            for blk in range(2):
                nc.vector.tensor_reduce(
                    pooled_d1d[:, blk, b:b + 1],
                    cur[:, blk, :].rearrange("p (l t) -> p l t", l=NL),
                    AX.XY, ALU.add)

            # ---- stage D/E: cosine-similarity maps (6 channels) ----
            sim_all = img.tile([96, 6 * T], F32R, tag="sim")
            xsq = img.tile([128, 2, T], F32, tag="xsq")
            nrm = img.tile([1, T], F32, tag="nrm")
            den = img.tile([96, T], F32, tag="den")
            for l in range(6):
                Xl = feats if l < 3 else xmix
                lo = (l % 3) * T
                gp = psG.tile([96, T], F32, tag="psg")
                for k in range(2):
                    nc.tensor.matmul(gp[:], Xl[:, k, lo:lo + T],
                                     Xl[:, k, lo:lo + T],
                                     start=(k == 0), stop=(k == 1))
                for k in range(2):
                    nc.vector.tensor_tensor(xsq[:, k, :], Xl[:, k, lo:lo + T],
                                            Xl[:, k, lo:lo + T], ALU.mult)
                nps = psG.tile([1, T], F32, tag="psg")
                for k in range(2):
                    nc.tensor.matmul(nps[:], ones_t[:, 0:1], xsq[:, k, :],
                                     start=(k == 0), stop=(k == 1))
                nc.scalar.activation(nrm[:], nps[:], AF.Sqrt)
                op = psG.tile([96, T], F32, tag="psg")
                nc.tensor.matmul(op[:], nrm[0:1, :], nrm[0:1, :],
                                 start=True, stop=True)
                nc.vector.tensor_scalar(den[:], op[:], EPS_COS, None, ALU.max)
                nc.vector.reciprocal(den[:], den[:])
                nc.vector.tensor_tensor(sim_all[:, l * T:(l + 1) * T], gp[:],
                                        den[:], ALU.mult)

            # ---- stage F: P54 im2col + stem conv ----
            p54 = scr.tile([54, FLW], F32R, tag="scr")
            zero_pads(p54[0:54, :], 54)
            nc.vector.tensor_copy(p54[0:54, BASE:BASE + S], zero_t[0:54, :])
            nc.vector.tensor_copy(p54[0:54, BASE + 95 * S:BASE + 96 * S],
                                  zero_t[0:54, :])
            brows = p54[0:54, BASE:BASE + T * S].rearrange("p (h s) -> p h s",
                                                           h=T)
            zc54 = zero_t[0:54, 0:T].rearrange("p (h o) -> p h o", h=T)
            nc.vector.tensor_copy(brows[:, :, 1:2], zc54)
            nc.vector.tensor_copy(brows[:, :, S - 2:S - 1], zc54)
            p54v = p54[0:54, BASE:BASE + T * S].rearrange("p (h s) -> p h s",
                                                          h=T)
            for l in range(6):
                for dh in range(3):
                    for dw in range(3):
                        row = l * 9 + dh * 3 + dw
                        h_lo, h_hi = max(0, 1 - dh), min(T - 1, T - dh)
                        w_lo, w_hi = max(1, 2 - dw), min(T, T + 1 - dw)
                        nh, nwd = h_hi - h_lo + 1, w_hi - w_lo + 1
                        nc.sync.dma_start(
                            p54v[row:row + 1, h_lo:h_lo + nh, w_lo:w_lo + nwd],
                            sim_all[h_lo + dh - 1:h_lo + dh - 1 + nh,
                                    l * T + w_lo + dw - 2:
                                    l * T + w_lo + dw - 2 + nwd])

            def interior(ap2d, ct):
                """(4, 96) interior view of a flat-layout 392-column tile."""
                return ap2d.rearrange("p (r s) -> p r s", r=4)[:, :, 1:1 + T]

            for m in range(2):
                for ct in range(NCT):
                    pc = psC.tile([128, COLT], F32, tag="psc")
                    nc.tensor.matmul(pc[:], stemw_t[:, m * 128:(m + 1) * 128],
                                     p54[:, BASE + ct * COLT:
                                         BASE + (ct + 1) * COLT],
                                     start=True, stop=True)
                    nc.scalar.activation(
                        interior(h_t[:, m, BASE + ct * COLT:
                                     BASE + (ct + 1) * COLT], ct),
                        interior(pc[:], ct),
                        AF.Relu, bias=spc("b_st", m), scale=spc("s_st", m))

            # ---- residual block: conv1 -> r, conv2 -> +h -> relu -> sums ----
            def conv3x3(conv_idx, rhs_fn, epi):
                for m in range(2):
                    wt_ = wst.tile([128, 2, 9, 128], F32R, tag="wst")
                    nc.sync.dma_start(wt_[:], convw_d[conv_idx, m])
                    for ct in range(NCT):
                        pc = psC.tile([128, COLT], F32, tag="psc")
                        t0, t1 = ct * COLT, (ct + 1) * COLT
                        first = True
                        for ti, (dh, dw) in enumerate(TAPS):
                            sh = (dh - 1) * S + (dw - 1)
                            a = max(t0, S if dh == 0 else 0)
                            bnd = min(t1, (T - 1) * S if dh == 2 else T * S)
                            if a >= bnd:
                                continue
                            for k in range(2):
                                nc.tensor.matmul(
                                    pc[:, a - t0:bnd - t0],
                                    wt_[:, k, dh * 3 + dw, :],
                                    rhs_fn(k, BASE + a + sh, BASE + bnd + sh),
                                    start=first,
                                    stop=(ti == len(TAPS) - 1 and k == 1))
                                first = False
                        epi(m, ct, pc)

            r_t = [scr.tile([128, FLW], F32R, tag="scr", name=f"r{_k}")
                   for _k in range(2)]
            for blk in range(2):
                zero_pads(r_t[blk][:], 128)

            def epi1(m, ct, pc):
                nc.scalar.activation(
                    interior(r_t[m][0:128, BASE + ct * COLT:
                                    BASE + (ct + 1) * COLT], ct),
                    interior(pc[:], ct),
                    AF.Relu, bias=spc("b_b1", m), scale=spc("s_b1", m))

            conv3x3(0, lambda k, lo, hi: h_t[:, k, lo:hi], epi1)

            dmacc = img.tile([128, 2, NCT], F32, tag="dmacc")

            def epi2(m, ct, pc):
                t1e = ep.tile([128, COLT], F32, tag="t1e")
                nc.vector.tensor_scalar(t1e[:], pc[:], spc("s_b2", m),
                                        spc("b_b2", m), ALU.mult, ALU.add)
                nc.vector.tensor_tensor(
                    t1e[:], t1e[:],
                    h_t[:, m, BASE + ct * COLT:BASE + (ct + 1) * COLT], ALU.add)
                rl = ep.tile([128, 4, T], F32, tag="rl")
                nc.scalar.activation(rl[:], interior(t1e[:], ct), AF.Relu,
                                     accum_out=dmacc[:, m, ct:ct + 1])

            conv3x3(1, lambda k, lo, hi: r_t[k][0:128, lo:hi], epi2)

            for blk in range(2):
                nc.vector.tensor_reduce(pooled_dm[:, blk, b:b + 1],
                                        dmacc[:, blk, :], AX.X, ALU.add)

        # ---- SE + fuse head, batched over the core's images (N=IPC) ----
        d1dm = fixed.tile([128, 2, IPC], F32)
        dmm = fixed.tile([128, 2, IPC], F32)
        for blk in range(2):
            nc.vector.tensor_scalar(d1dm[:, blk, :], pooled_d1d[:, blk, :],
                                    1.0 / (NL * T), None, ALU.mult)
            nc.vector.tensor_scalar(dmm[:, blk, :], pooled_dm[:, blk, :],
                                    1.0 / (T * T), None, ALU.mult)

        def se(pool_t, w1_t, w2_t, name):
            ps1 = psG.tile([64, IPC], F32, tag="psg")
            for k in range(2):
                nc.tensor.matmul(ps1[:], w1_t[:, k, :], pool_t[:, k, :],
                                 start=(k == 0), stop=(k == 1))
            h1 = fixed.tile([64, IPC], F32, tag=f"seh_{name}")
            nc.scalar.activation(h1[:], ps1[:], AF.Relu)
            out = fixed.tile([128, 2, IPC], F32, tag=f"seo_{name}")
            for m in range(2):
                ps2 = psG.tile([128, IPC], F32, tag="psg")
                nc.tensor.matmul(ps2[:], w2_t[:, m * 128:(m + 1) * 128], h1[:],
                                 start=True, stop=True)
                nc.scalar.activation(out[:, m, :], ps2[:], AF.Sigmoid)
            return out

        s1_t = se(d1dm, se1w1_t, se1w2_t, "se1")
        s2_t = se(dmm, se2w1_t, se2w2_t, "se2")
        i2d = fixed.tile([128, 2, IPC], F32)
        d2i = fixed.tile([128, 2, IPC], F32)
        for blk in range(2):
            nc.vector.tensor_tensor(i2d[:, blk, :], dmm[:, blk, :],
                                    s1_t[:, blk, :], ALU.mult)
            nc.vector.tensor_tensor(d2i[:, blk, :], d1dm[:, blk, :],
                                    s2_t[:, blk, :], ALU.mult)

        hf = fixed.tile([128, 2, IPC], F32)
        for m in range(2):
            psf = psG.tile([128, IPC], F32, tag="psg")
            for k in range(4):
                rhs = i2d[:, k, :] if k < 2 else d2i[:, k - 2, :]
                nc.tensor.matmul(psf[:], fusew_t[:, k, m * 128:(m + 1) * 128],
                                 rhs, start=(k == 0), stop=(k == 3))
            nc.scalar.activation(hf[:, m, :], psf[:], AF.Relu,
                                 bias=spc("fuse_b", m))

        for b in range(IPC):
            for m in range(2):
                nc.sync.dma_start(out_d[b, m * 128:(m + 1) * 128],
                                  hf[:, m, b:b + 1])

    nc.finalize()
    return nc


_NC_CACHE = None


def _get_program():
    global _NC_CACHE
    if _NC_CACHE is None:
        _NC_CACHE = build_program()
    return _NC_CACHE


def _prep_params(params):
    p = {k: np.asarray(v, np.float32) for k, v in params.items()}
    inv = 1.0 / np.sqrt(1.0 + EPS_BN)

    sp = np.zeros((256, NSP), np.float32)

    def setcol(name, vec):
        sp[:, _col(name)] = vec

    for i in range(3):
        for j in range(3):
            setcol(f"tw{i}{j}", p["t_w"][i, :, 0, j])
        setcol(f"tb{i}", p["t_b"][i])
    setcol("s_pre", p["mx_pre_g"] * inv)
    setcol("b_pre", p["mx_pre_b"])
    s0 = p["mx_g0"] * inv
    setcol("s_mx0", s0)
    setcol("b_mx0", p["mx_b0"] * s0 + p["mx_bb0"])
    s1_ = p["mx_g1"] * inv
    setcol("s_mx1", s1_)
    setcol("b_mx1", p["mx_b1"] * s1_ + p["mx_bb1"])
    for j in range(2):
        for t_ in range(3):
            setcol(f"edw{j}{t_}", p["enc_dw_w"][j, :, 0, t_])
        setcol(f"edb{j}", p["enc_dw_b"][j])
        sj = p["enc_g"][j] * inv
        setcol(f"s_enc{j}", sj)
        setcol(f"b_enc{j}", p["enc_pw_b"][j] * sj + p["enc_b"][j])
    sst = p["stem_g"] * inv
    setcol("s_st", sst)
    setcol("b_st", p["stem_b"] * sst + p["stem_bb"])
    sb1 = p["blk_g1"] * inv
    setcol("s_b1", sb1)
    setcol("b_b1", p["blk_b1"] * sb1 + p["blk_bb1"])
    sb2 = p["blk_g2"] * inv
    setcol("s_b2", sb2)
    setcol("b_b2", p["blk_b2"] * sb2 + p["blk_bb2"])
    setcol("fuse_b", p["fuse_b"])
    sp = np.ascontiguousarray(sp.reshape(2, 128, NSP).transpose(1, 0, 2))

    def lhsT_blocks(w, nk):
        wt = w.T.reshape(nk, 128, w.shape[0]).transpose(1, 0, 2)
        return np.ascontiguousarray(wt, np.float32)

    mxw0 = lhsT_blocks(p["mx_w0"][:, :, 0], 2)
    mxw1 = lhsT_blocks(p["mx_w1"][:, :, 0], 2)
    encw0 = lhsT_blocks(p["enc_pw_w"][0][:, :, 0], 2)
    encw1 = lhsT_blocks(p["enc_pw_w"][1][:, :, 0], 2)
    fusew = lhsT_blocks(p["fuse_w"], 4)
    se1w1 = lhsT_blocks(p["se1_w1"], 2)
    se1w2 = np.ascontiguousarray(p["se1_w2"].T, np.float32)
    se2w1 = lhsT_blocks(p["se2_w1"], 2)
    se2w2 = np.ascontiguousarray(p["se2_w2"].T, np.float32)

    convw = np.zeros((2, 2, 128, 2, 9, 128), np.float32)
    for ci, key in enumerate(("blk_w1", "blk_w2")):
        w = p[key].reshape(256, 256, 9)             # [cout, cin, tap]
        wt = w.transpose(1, 0, 2)                   # [cin, cout, tap]
        for m in range(2):
            blkw = wt[:, m * 128:(m + 1) * 128, :]  # [cin, 128, tap]
            convw[ci, m] = blkw.reshape(2, 128, 128, 9).transpose(1, 0, 3, 2)

    stemw = np.ascontiguousarray(
        p["stem_w"].transpose(1, 2, 3, 0).reshape(54, 256), np.float32)

    return dict(convw=convw, stemw=stemw, mxw0=mxw0, mxw1=mxw1, encw0=encw0,
                encw1=encw1, fusew=fusew, se1w1=se1w1, se1w2=se1w2,
                se2w1=se2w1, se2w2=se2w2, sp=sp)


def kernel(x0, x1, x2, nw, params, _trace=False):
    from concourse.bass_utils import run_bass_kernel_spmd

    x0 = np.asarray(x0, np.float32)
    x1 = np.asarray(x1, np.float32)
    x2 = np.asarray(x2, np.float32)
    nw = int(nw)
    wmap = _prep_params(params)
    xall = np.stack([x0, x1, x2])  # [3, 32, 256, 96]

    nc = _get_program()
    in_maps = []
    for c in range(NCORES):
        m = dict(wmap)
        m["x"] = np.ascontiguousarray(xall[:, c * IPC:(c + 1) * IPC])
        in_maps.append(m)
    res = run_bass_kernel_spmd(nc, in_maps, list(range(NCORES)), trace=_trace)
    hf = np.concatenate([res.results[c]["out"] for c in range(NCORES)], axis=0)
    out = hf.reshape(-1, DIM, nw)
    if _trace:
        return out, res
    return out
